# revision 31
# baseline (speedup 1.0000x reference)
"""Trainium2 Bass kernel for nn_MultiHeadAttention (B=2, S=2048, D=1024, H=16).

Sharding (8 cores): data-parallel over batch (2) x tensor-parallel over
head groups (4 groups of 4 heads). Core c handles batch c//4, heads
4*(c%4) .. 4*(c%4)+3.  Each core computes the full attention for its
heads plus its slice of the output projection; the host sums the 4
partial output projections per batch and adds bo.

On-chip layouts (per core):
  qT, kT  [256 feat, 2048 seq]   (features on partitions)
  v       [2048 keys, 4*65]      (per head: 64 feats + ones column)
  scoresT [keys, queries] tiles -> exp on the scalar engine with the
          1/sqrt(64) scale fused (max-subtract skipped: softmax is
          shift invariant and scores are O(1) here)
  ctxT    [65, queries] accumulated over key tiles; row 64 = sum of exp
          (from the ones column) -> broadcast -> reciprocal -> scale.
All matmuls run in bf16 with fp32 PSUM accumulation; inputs are cast to
bf16 on the host (halves HBM traffic, enables fast weight loads).

Schedule: the q/k/v projections are emitted as <=4-matmul "filler
granules" popped between attention chunks so they hide in the scalar
engine (exp) bound attention phase.  Tile dependencies are trace-order
based, so every producer granule pops before its first consumer is
emitted.  ctx matmuls for j==0 are deferred (exp tiles buffered) until
the interleaved v projection has produced the needed v tiles.
"""

import sys

for _p in ("/opt/trn_rl_repo",):
    if _p not in sys.path:
        sys.path.insert(0, _p)

from contextlib import ExitStack

import ml_dtypes
import numpy as np

import concourse.bass as bass
import concourse.tile as tile
from concourse import bacc, mybir
from concourse.bass_utils import run_bass_kernel_spmd

B, S, D, H = 2, 2048, 1024, 16
HD = D // H            # 64 head dim
NG = 4                 # head groups (cores per batch)
NHC = H // NG          # 4 heads per core
FS = NHC * HD          # 256 features per core
P = 128
DK = D // P            # 8 contraction tiles for projections
SK = S // P            # 16 key tiles
NQ = S // 512          # 4 query chunks
FK = FS // P           # 2 feature tiles for qT/kT/ctxT
VW = HD + 1            # v feats + ones column

f32 = mybir.dt.float32
bf16 = mybir.dt.bfloat16
EXP = mybir.ActivationFunctionType.Exp
CHUNKS = (2,) * 8   # key tiles per exp chunk (16 total)


def _emit(ctx: ExitStack, tc, nc, io):
    QT, KT, VT, WqT, WkT, WvT, WoT, bq, bk, bv, OUTP = io

    xt = ctx.enter_context(tc.tile_pool(name="xt", bufs=24))
    wp = ctx.enter_context(tc.tile_pool(name="wp", bufs=1))
    per = ctx.enter_context(tc.tile_pool(name="per", bufs=1))
    exp = ctx.enter_context(tc.tile_pool(name="exp", bufs=18))
    nrm = ctx.enter_context(tc.tile_pool(name="nrm", bufs=2))
    ctxp = ctx.enter_context(tc.tile_pool(name="ctxp", bufs=2))
    outp = ctx.enter_context(tc.tile_pool(name="outp", bufs=4))
    sc_ps = ctx.enter_context(tc.tile_pool(name="sc_ps", bufs=3, space="PSUM"))
    acc_ps = ctx.enter_context(tc.tile_pool(name="acc_ps", bufs=2, space="PSUM"))

    # ---- weights / biases (persistent) ----
    wq = [wp.tile([P, FS], bf16, tag=f"wq{k}", name=f"wq{k}") for k in range(DK)]
    wk = [wp.tile([P, FS], bf16, tag=f"wk{k}", name=f"wk{k}") for k in range(DK)]
    wv = [wp.tile([P, FS], bf16, tag=f"wv{k}", name=f"wv{k}") for k in range(DK)]
    wo = [wp.tile([P, D], bf16, tag=f"wo{f}", name=f"wo{f}") for f in range(FK)]
    for k in range(DK):
        nc.sync.dma_start(wk[k][:], WkT[k * P:(k + 1) * P, :])
    bq_t = [wp.tile([P, 1], f32, tag=f"bq{f}", name=f"bqt{f}") for f in range(FK)]
    bk_t = [wp.tile([P, 1], f32, tag=f"bk{f}", name=f"bkt{f}") for f in range(FK)]
    for f in range(FK):
        nc.sync.dma_start(bq_t[f][:], bq[f * P:(f + 1) * P, :])
        nc.sync.dma_start(bk_t[f][:], bk[f * P:(f + 1) * P, :])
    bv_t = wp.tile([P, FS], f32, tag="bv")
    nc.sync.dma_start(bv_t[:], bv.to_broadcast((P, FS)))
    ones_t = wp.tile([P, NHC], f32, tag="ones")
    nc.vector.memset(ones_t[:], 1.0)

    # ---- persistent activations ----
    kT = [per.tile([P, S], bf16, tag=f"kT{f}", name=f"kTs{f}") for f in range(FK)]
    qT = [per.tile([P, S], bf16, tag=f"qT{f}", name=f"qTs{f}") for f in range(FK)]
    vsb = [per.tile([P, NHC * VW], bf16, tag=f"v{t}", name=f"vs{t}")
           for t in range(SK)]

    # ---- input streaming: [128, 1024] bf16 tiles ----
    def load_half(src, hf, eng=None):
        tiles = {}
        for k in range(DK):
            t = xt.tile([P, 1024], bf16, tag="xt", name="xtile")
            (eng or nc.sync).dma_start(t[:], src[k * P:(k + 1) * P,
                                                 hf * 1024:(hf + 1) * 1024])
            tiles[k] = t
        return tiles

    def proj_cols(src_tiles, w, b_t, dst, ncol):
        # dst[f][:, ncol*512:+512] = (W_slice @ X^T + b)
        off = (ncol * 512) % 1024
        for f in range(FK):
            ps = acc_ps.tile([P, 512], f32, tag="acc")
            for k in range(DK):
                nc.tensor.matmul(
                    ps[:],
                    w[k][:, f * P:(f + 1) * P],
                    src_tiles[k][:, off:off + 512],
                    start=(k == 0), stop=(k == DK - 1),
                )
            nc.vector.tensor_scalar_add(
                dst[f][:, ncol * 512:(ncol + 1) * 512], ps[:], b_t[f][:])

    # ---- emission order tuned for overlap ----
    kt_h = [load_half(KT, 0), load_half(KT, 1)]
    for k in range(DK):
        nc.gpsimd.dma_start(wq[k][:], WqT[k * P:(k + 1) * P, :])
    qt_h0 = load_half(QT, 0, nc.gpsimd)
    vt_h = [load_half(VT, 0, nc.scalar), load_half(VT, 1, nc.scalar)]
    for k in range(DK):
        nc.sync.dma_start(wv[k][:], WvT[k * P:(k + 1) * P, :])
    for f in range(FK):
        nc.sync.dma_start(wo[f][:], WoT[f * P:(f + 1) * P, :])
    qt_h1 = load_half(QT, 1, nc.gpsimd)

    # HAM pre-warm: ~5us of dependency-free matmuls while the first DMAs
    # land, so real matmuls start at 2.4 GHz instead of 1.2 GHz.
    warm_sb = wp.tile([P, 16], f32, tag="warm")
    nc.vector.memset(warm_sb[:], 0.0)
    warm_ps = acc_ps.tile([16, 16], f32, tag="acc", name="warmps")
    for _ in range(100):
        nc.tensor.matmul(warm_ps[:], warm_sb[:, 0:16], warm_sb[:],
                         start=True, stop=True)
    warm_ex = wp.tile([P, 16], bf16, tag="warmex")
    nc.scalar.activation(warm_ex[:], warm_sb[:], EXP, scale=0.125)

    proj_cols(kt_h[0], wk, bk_t, kT, 0)
    proj_cols(qt_h0, wq, bq_t, qT, 0)

    # ---- filler granules (<=4 matmuls each) ----
    def proj_granules(src_tiles, w, b_t, dst, ncol, f):
        off = (ncol * 512) % 1024
        st = {}

        def g0():
            st["ps"] = acc_ps.tile([P, 512], f32, tag="acc", name="pps")
            for k in range(4):
                nc.tensor.matmul(
                    st["ps"][:], w[k][:, f * P:(f + 1) * P],
                    src_tiles[k][:, off:off + 512],
                    start=(k == 0), stop=False)

        def g1():
            for k in range(4, DK):
                nc.tensor.matmul(
                    st["ps"][:], w[k][:, f * P:(f + 1) * P],
                    src_tiles[k][:, off:off + 512],
                    start=False, stop=(k == DK - 1))
            nc.vector.tensor_scalar_add(
                dst[f][:, ncol * 512:(ncol + 1) * 512], st["ps"][:], b_t[f][:])
        return [g0, g1]

    def vproj_granules(t):
        st = {}

        def g0():
            st["ps"] = acc_ps.tile([P, FS], f32, tag="acc", name="vps")
            for k in range(4):
                nc.tensor.matmul(
                    st["ps"][:],
                    vt_h[t * P // 1024][k][:, (t * P) % 1024:(t * P) % 1024 + P],
                    wv[k][:], start=(k == 0), stop=False)

        def g1():
            for k in range(4, DK):
                nc.tensor.matmul(
                    st["ps"][:],
                    vt_h[t * P // 1024][k][:, (t * P) % 1024:(t * P) % 1024 + P],
                    wv[k][:], start=False, stop=(k == DK - 1))
            for h in range(NHC):
                nc.vector.tensor_add(
                    vsb[t][:, h * VW:h * VW + HD],
                    st["ps"][:, h * HD:(h + 1) * HD],
                    bv_t[:, h * HD:(h + 1) * HD])
            nc.vector.tensor_copy(vsb[t][:, HD:NHC * VW:VW], ones_t[:])
        return [g0, g1]

    def outproj_unit(j, ctxn, mt, oc):
        def emit():
            ps = acc_ps.tile([P, 512], f32, tag="acc", name="ops")
            for f in range(FK):
                nc.tensor.matmul(
                    ps[:], ctxn[f][:, mt * P:(mt + 1) * P],
                    wo[f][:, oc * 512:(oc + 1) * 512],
                    start=(f == 0), stop=(f == FK - 1))
            ob = outp.tile([P, 512], f32, tag="ob")
            nc.vector.tensor_copy(ob[:], ps[:])
            nc.sync.dma_start(
                OUTP[j * 512 + mt * P: j * 512 + (mt + 1) * P,
                     oc * 512:(oc + 1) * 512], ob[:])
        return emit

    def outproj_units(j, ctxn):
        return [outproj_unit(j, ctxn, mt, oc)
                for mt in range(4) for oc in range(2)]

    # NOTE: Tile dependencies are trace-order based -- every producer must
    # be EMITTED before its consumer.  All projection fillers therefore pop
    # inside j==0 (before any j>=1 instruction is traced).
    j0_fill = []
    for ncol in (1, 2, 3):
        for f in range(FK):
            j0_fill += proj_granules(kt_h[ncol // 2], wk, bk_t, kT, ncol, f)
    n_vp_before = len(j0_fill)  # 12 kT granules popped first
    for t in range(SK):
        j0_fill += vproj_granules(t)
    for f in range(FK):
        j0_fill += proj_granules(qt_h0, wq, bq_t, qT, 1, f)
    jn_fill = {1: [], 2: [], 3: []}
    for f in range(FK):
        jn_fill[1] += proj_granules(qt_h1, wq, bq_t, qT, 2, f)
    for f in range(FK):
        jn_fill[2] += proj_granules(qt_h1, wq, bq_t, qT, 3, f)

    NCH = len(CHUNKS)  # 8 chunks per head pass

    def norm_head(h, ctx_ps, ctxn):
        fq, rq = divmod(h * HD, P)
        sm = nrm.tile([1, 512], f32, tag="sm")
        nc.vector.tensor_copy(sm[:], ctx_ps[HD:HD + 1, :])
        sb = nrm.tile([HD, 512], f32, tag="sb")
        nc.gpsimd.partition_broadcast(sb[:], sm[:])
        rb = nrm.tile([HD, 512], f32, tag="rb")
        rs = nrm.tile([HD, 512], f32, tag="rs")
        nc.vector.reciprocal_approx_accurate(rb[:], sb[:], rs[:])
        nc.vector.tensor_mul(ctxn[fq][rq:rq + HD, :], ctx_ps[0:HD, :], rb[:])

    prev = None
    for j in range(NQ):
        queue = list(j0_fill) if j == 0 else list(jn_fill[j])
        n_early = len(queue)
        if prev is not None:
            queue += outproj_units(prev[0], prev[1])
        pops_per_chunk = 2 if j == 0 else 1
        skip_chunks = 0 if j == 0 else 2  # let the norm chain drain first
        qi = 0
        ctxn = [ctxp.tile([P, 512], bf16, tag=f"ctxn{f}", name=f"ctxn{f}")
                for f in range(FK)]
        ctx_ps_h = {}
        exb = {}           # (h, c) -> ex tile (deferred-consumption buffers)
        pending = []       # ctx chunk work: (h, c)
        done_ctx = {h: 0 for h in range(NHC)}

        def emit_ctx(h, c):
            if h not in ctx_ps_h:
                ctx_ps_h[h] = acc_ps.tile([VW, 512], f32, tag="acc",
                                          name=f"ctxps{h}")
            cp = ctx_ps_h[h]
            ex = exb.pop((h, c))
            for t in range(2):
                kt2 = 2 * c + t
                nc.tensor.matmul(
                    cp[:], vsb[kt2][:, h * VW:(h + 1) * VW],
                    ex[:, t * 512:(t + 1) * 512],
                    start=(kt2 == 0), stop=(kt2 == SK - 1))
            done_ctx[h] += 1
            if done_ctx[h] == NCH:
                norm_head(h, cp, ctxn)

        def vsb_ready_upto(popped):
            vp = max(0, popped - n_vp_before) if j == 0 else 10 ** 9
            return vp // 2 - 1 if j == 0 else 10 ** 9

        for h in range(NHC):
            fq, rq = divmod(h * HD, P)
            qv = qT[fq][rq:rq + HD, j * 512:(j + 1) * 512]
            for c in range(NCH):
                sc = sc_ps.tile([P, 2 * 512], f32, tag="sc")
                for t in range(2):
                    kt2 = 2 * c + t
                    nc.tensor.matmul(
                        sc[:, t * 512:(t + 1) * 512],
                        kT[fq][rq:rq + HD, kt2 * P:(kt2 + 1) * P],
                        qv, start=True, stop=True)
                ex = exp.tile([P, 2 * 512], bf16, tag="ex")
                nc.scalar.activation(ex[:], sc[:], EXP,
                                     scale=1.0 / (HD ** 0.5))
                exb[(h, c)] = ex
                pending.append((h, c))
                gchunk = h * NCH + c
                if gchunk >= skip_chunks or qi < n_early:
                    for _ in range(pops_per_chunk):
                        if qi < len(queue):
                            queue[qi]()
                            qi += 1
                tmax = vsb_ready_upto(qi)
                while pending and 2 * pending[0][1] + 1 <= tmax:
                    emit_ctx(*pending.pop(0))
        while qi < len(queue):
            queue[qi]()
            qi += 1
        while pending:
            emit_ctx(*pending.pop(0))
        prev = (j, ctxn)
    for u in outproj_units(prev[0], prev[1]):
        u()


_CACHE = {}


def _build():
    if "nc" in _CACHE:
        return _CACHE["nc"]
    nc = bacc.Bacc("TRN2", target_bir_lowering=False, debug=False)
    QT = nc.dram_tensor("QT", [D, S], bf16, kind="ExternalInput").ap()
    KT = nc.dram_tensor("KT", [D, S], bf16, kind="ExternalInput").ap()
    VT = nc.dram_tensor("VT", [D, S], bf16, kind="ExternalInput").ap()
    WqT = nc.dram_tensor("WqT", [D, FS], bf16, kind="ExternalInput").ap()
    WkT = nc.dram_tensor("WkT", [D, FS], bf16, kind="ExternalInput").ap()
    WvT = nc.dram_tensor("WvT", [D, FS], bf16, kind="ExternalInput").ap()
    WoT = nc.dram_tensor("WoT", [FS, D], bf16, kind="ExternalInput").ap()
    bq = nc.dram_tensor("bq", [FS, 1], f32, kind="ExternalInput").ap()
    bk = nc.dram_tensor("bk", [FS, 1], f32, kind="ExternalInput").ap()
    bv = nc.dram_tensor("bv", [1, FS], f32, kind="ExternalInput").ap()
    OUTP = nc.dram_tensor("OUTP", [S, D], f32, kind="ExternalOutput").ap()
    with tile.TileContext(nc) as tc, ExitStack() as ctx:
        _emit(ctx, tc, nc, (QT, KT, VT, WqT, WkT, WvT, WoT, bq, bk, bv, OUTP))
    nc.compile()
    _CACHE["nc"] = nc
    return nc


def _in_maps(Q, K, V, Wq, bq, Wk, bk, Wv, bv, Wo, bo):
    bf = ml_dtypes.bfloat16

    def cT(a):  # contiguous bf16 transpose
        return np.ascontiguousarray(np.asarray(a).T).astype(bf)

    QTb = [cT(Q[b]) for b in range(B)]
    KTb = [cT(K[b]) for b in range(B)]
    VTb = [cT(V[b]) for b in range(B)]
    c = np.ascontiguousarray
    maps = []
    for core in range(8):
        b, g = divmod(core, NG)
        sl = slice(g * FS, (g + 1) * FS)
        maps.append({
            "QT": QTb[b], "KT": KTb[b], "VT": VTb[b],
            "WqT": cT(Wq[sl, :]), "WkT": cT(Wk[sl, :]),
            "WvT": cT(Wv[sl, :]), "WoT": cT(Wo[:, sl]),
            "bq": c(np.asarray(bq)[sl].reshape(FS, 1)),
            "bk": c(np.asarray(bk)[sl].reshape(FS, 1)),
            "bv": c(np.asarray(bv)[sl].reshape(1, FS)),
        })
    return maps


def kernel(Q, K, V, Wq, bq, Wk, bk, Wv, bv, Wo, bo):
    nc = _build()
    maps = _in_maps(Q, K, V, Wq, bq, Wk, bk, Wv, bv, Wo, bo)
    res = run_bass_kernel_spmd(nc, maps, core_ids=list(range(8)))
    out = np.empty((B, S, D), np.float32)
    for b in range(B):
        acc = res.results[b * NG]["OUTP"].astype(np.float32)
        for g in range(1, NG):
            acc = acc + res.results[b * NG + g]["OUTP"]
        out[b] = acc + np.asarray(bo, np.float32)[None, :]
    return out


# revision 32
# speedup vs baseline: 1.0147x; 1.0147x over previous
"""Trainium2 Bass kernel for nn_MultiHeadAttention (B=2, S=2048, D=1024, H=16).

Sharding (8 cores): data-parallel over batch (2) x tensor-parallel over
head groups (4 groups of 4 heads). Core c handles batch c//4, heads
4*(c%4) .. 4*(c%4)+3.  Each core computes the full attention for its
heads plus its slice of the output projection; the host sums the 4
partial output projections per batch and adds bo.

On-chip layouts (per core):
  qT, kT  [256 feat, 2048 seq]   (features on partitions)
  v       [2048 keys, 4*65]      (per head: 64 feats + ones column)
  scoresT [keys, queries] tiles -> exp on the scalar engine with the
          1/sqrt(64) scale fused (max-subtract skipped: softmax is
          shift invariant and scores are O(1) here)
  ctxT    [65, queries] accumulated over key tiles; row 64 = sum of exp
          (from the ones column) -> broadcast -> reciprocal -> scale.
All matmuls run in bf16 with fp32 PSUM accumulation; inputs are cast to
bf16 on the host (halves HBM traffic, enables fast weight loads).

Schedule: the q/k/v projections are emitted as <=4-matmul "filler
granules" popped between attention chunks so they hide in the scalar
engine (exp) bound attention phase.  Tile dependencies are trace-order
based, so every producer granule pops before its first consumer is
emitted.  ctx matmuls for j==0 are deferred (exp tiles buffered) until
the interleaved v projection has produced the needed v tiles.
"""

import sys

for _p in ("/opt/trn_rl_repo",):
    if _p not in sys.path:
        sys.path.insert(0, _p)

from contextlib import ExitStack

import ml_dtypes
import numpy as np

import concourse.bass as bass
import concourse.tile as tile
from concourse import bacc, mybir
from concourse.bass_utils import run_bass_kernel_spmd

B, S, D, H = 2, 2048, 1024, 16
HD = D // H            # 64 head dim
NG = 4                 # head groups (cores per batch)
NHC = H // NG          # 4 heads per core
FS = NHC * HD          # 256 features per core
P = 128
DK = D // P            # 8 contraction tiles for projections
SK = S // P            # 16 key tiles
NQ = S // 512          # 4 query chunks
FK = FS // P           # 2 feature tiles for qT/kT/ctxT
VW = HD + 1            # v feats + ones column

f32 = mybir.dt.float32
bf16 = mybir.dt.bfloat16
EXP = mybir.ActivationFunctionType.Exp
CHUNKS = (2,) * 8   # key tiles per exp chunk (16 total)


def _emit(ctx: ExitStack, tc, nc, io):
    QT, KT, VT, WqT, WkT, WvT, WoT, bq, bk, bv, OUTP = io

    xt = ctx.enter_context(tc.tile_pool(name="xt", bufs=24))
    wp = ctx.enter_context(tc.tile_pool(name="wp", bufs=1))
    per = ctx.enter_context(tc.tile_pool(name="per", bufs=1))
    exp = ctx.enter_context(tc.tile_pool(name="exp", bufs=18))
    nrm = ctx.enter_context(tc.tile_pool(name="nrm", bufs=2))
    ctxp = ctx.enter_context(tc.tile_pool(name="ctxp", bufs=2))
    outp = ctx.enter_context(tc.tile_pool(name="outp", bufs=4))
    sc_ps = ctx.enter_context(tc.tile_pool(name="sc_ps", bufs=2, space="PSUM"))
    acc_ps = ctx.enter_context(tc.tile_pool(name="acc_ps", bufs=4, space="PSUM"))

    # ---- weights / biases (persistent) ----
    wq = [wp.tile([P, FS], bf16, tag=f"wq{k}", name=f"wq{k}") for k in range(DK)]
    wk = [wp.tile([P, FS], bf16, tag=f"wk{k}", name=f"wk{k}") for k in range(DK)]
    wv = [wp.tile([P, FS], bf16, tag=f"wv{k}", name=f"wv{k}") for k in range(DK)]
    wo = [wp.tile([P, D], bf16, tag=f"wo{f}", name=f"wo{f}") for f in range(FK)]
    for k in range(DK):
        nc.sync.dma_start(wk[k][:], WkT[k * P:(k + 1) * P, :])
    bq_t = [wp.tile([P, 1], f32, tag=f"bq{f}", name=f"bqt{f}") for f in range(FK)]
    bk_t = [wp.tile([P, 1], f32, tag=f"bk{f}", name=f"bkt{f}") for f in range(FK)]
    for f in range(FK):
        nc.sync.dma_start(bq_t[f][:], bq[f * P:(f + 1) * P, :])
        nc.sync.dma_start(bk_t[f][:], bk[f * P:(f + 1) * P, :])
    bv_t = wp.tile([P, FS], f32, tag="bv")
    nc.sync.dma_start(bv_t[:], bv.to_broadcast((P, FS)))
    ones_t = wp.tile([P, NHC], f32, tag="ones")
    nc.vector.memset(ones_t[:], 1.0)

    # ---- persistent activations ----
    kT = [per.tile([P, S], bf16, tag=f"kT{f}", name=f"kTs{f}") for f in range(FK)]
    qT = [per.tile([P, S], bf16, tag=f"qT{f}", name=f"qTs{f}") for f in range(FK)]
    vsb = [per.tile([P, NHC * VW], bf16, tag=f"v{t}", name=f"vs{t}")
           for t in range(SK)]

    # ---- input streaming: [128, 1024] bf16 tiles ----
    def load_half(src, hf, eng=None):
        tiles = {}
        for k in range(DK):
            t = xt.tile([P, 1024], bf16, tag="xt", name="xtile")
            (eng or nc.sync).dma_start(t[:], src[k * P:(k + 1) * P,
                                                 hf * 1024:(hf + 1) * 1024])
            tiles[k] = t
        return tiles

    def proj_cols(src_tiles, w, b_t, dst, ncol):
        # dst[f][:, ncol*512:+512] = (W_slice @ X^T + b)
        off = (ncol * 512) % 1024
        for f in range(FK):
            ps = acc_ps.tile([P, 512], f32, tag="acc")
            for k in range(DK):
                nc.tensor.matmul(
                    ps[:],
                    w[k][:, f * P:(f + 1) * P],
                    src_tiles[k][:, off:off + 512],
                    start=(k == 0), stop=(k == DK - 1),
                )
            nc.vector.tensor_scalar_add(
                dst[f][:, ncol * 512:(ncol + 1) * 512], ps[:], b_t[f][:])

    # ---- emission order tuned for overlap ----
    kt_h = [load_half(KT, 0), load_half(KT, 1)]
    for k in range(DK):
        nc.gpsimd.dma_start(wq[k][:], WqT[k * P:(k + 1) * P, :])
    qt_h0 = load_half(QT, 0, nc.gpsimd)
    vt_h = [load_half(VT, 0, nc.scalar), load_half(VT, 1, nc.scalar)]
    for k in range(DK):
        nc.sync.dma_start(wv[k][:], WvT[k * P:(k + 1) * P, :])
    for f in range(FK):
        nc.sync.dma_start(wo[f][:], WoT[f * P:(f + 1) * P, :])
    qt_h1 = load_half(QT, 1, nc.gpsimd)

    # HAM pre-warm: ~5us of dependency-free matmuls while the first DMAs
    # land, so real matmuls start at 2.4 GHz instead of 1.2 GHz.
    warm_sb = wp.tile([P, 16], f32, tag="warm")
    nc.vector.memset(warm_sb[:], 0.0)
    warm_ps = acc_ps.tile([16, 16], f32, tag="acc", name="warmps")
    for _ in range(100):
        nc.tensor.matmul(warm_ps[:], warm_sb[:, 0:16], warm_sb[:],
                         start=True, stop=True)
    warm_ex = wp.tile([P, 16], bf16, tag="warmex")
    nc.scalar.activation(warm_ex[:], warm_sb[:], EXP, scale=0.125)

    proj_cols(kt_h[0], wk, bk_t, kT, 0)
    proj_cols(qt_h0, wq, bq_t, qT, 0)

    # ---- filler granules (<=4 matmuls each) ----
    def proj_granules(src_tiles, w, b_t, dst, ncol, f):
        off = (ncol * 512) % 1024
        st = {}

        def g0():
            st["ps"] = acc_ps.tile([P, 512], f32, tag="acc", name="pps")
            for k in range(4):
                nc.tensor.matmul(
                    st["ps"][:], w[k][:, f * P:(f + 1) * P],
                    src_tiles[k][:, off:off + 512],
                    start=(k == 0), stop=False)

        def g1():
            for k in range(4, DK):
                nc.tensor.matmul(
                    st["ps"][:], w[k][:, f * P:(f + 1) * P],
                    src_tiles[k][:, off:off + 512],
                    start=False, stop=(k == DK - 1))
            nc.vector.tensor_scalar_add(
                dst[f][:, ncol * 512:(ncol + 1) * 512], st["ps"][:], b_t[f][:])
        return [g0, g1]

    def vproj_granules(t):
        st = {}

        def g0():
            st["ps"] = acc_ps.tile([P, FS], f32, tag="acc", name="vps")
            for k in range(4):
                nc.tensor.matmul(
                    st["ps"][:],
                    vt_h[t * P // 1024][k][:, (t * P) % 1024:(t * P) % 1024 + P],
                    wv[k][:], start=(k == 0), stop=False)

        def g1():
            for k in range(4, DK):
                nc.tensor.matmul(
                    st["ps"][:],
                    vt_h[t * P // 1024][k][:, (t * P) % 1024:(t * P) % 1024 + P],
                    wv[k][:], start=False, stop=(k == DK - 1))
            for h in range(NHC):
                nc.vector.tensor_add(
                    vsb[t][:, h * VW:h * VW + HD],
                    st["ps"][:, h * HD:(h + 1) * HD],
                    bv_t[:, h * HD:(h + 1) * HD])
            nc.vector.tensor_copy(vsb[t][:, HD:NHC * VW:VW], ones_t[:])
        return [g0, g1]

    def outproj_unit(j, ctxn, mt, oc):
        def emit():
            ps = acc_ps.tile([P, 512], f32, tag="acc", name="ops")
            for f in range(FK):
                nc.tensor.matmul(
                    ps[:], ctxn[f][:, mt * P:(mt + 1) * P],
                    wo[f][:, oc * 512:(oc + 1) * 512],
                    start=(f == 0), stop=(f == FK - 1))
            ob = outp.tile([P, 512], f32, tag="ob")
            nc.vector.tensor_copy(ob[:], ps[:])
            nc.sync.dma_start(
                OUTP[j * 512 + mt * P: j * 512 + (mt + 1) * P,
                     oc * 512:(oc + 1) * 512], ob[:])
        return emit

    def outproj_units(j, ctxn):
        return [outproj_unit(j, ctxn, mt, oc)
                for mt in range(4) for oc in range(2)]

    # NOTE: Tile dependencies are trace-order based -- every producer must
    # be EMITTED before its consumer.  All projection fillers therefore pop
    # inside j==0 (before any j>=1 instruction is traced).
    j0_fill = []
    for ncol in (1, 2, 3):
        for f in range(FK):
            j0_fill += proj_granules(kt_h[ncol // 2], wk, bk_t, kT, ncol, f)
    n_vp_before = len(j0_fill)  # 12 kT granules popped first
    for t in range(SK):
        j0_fill += vproj_granules(t)
    for f in range(FK):
        j0_fill += proj_granules(qt_h0, wq, bq_t, qT, 1, f)
    jn_fill = {1: [], 2: [], 3: []}
    for f in range(FK):
        jn_fill[1] += proj_granules(qt_h1, wq, bq_t, qT, 2, f)
    for f in range(FK):
        jn_fill[2] += proj_granules(qt_h1, wq, bq_t, qT, 3, f)

    NCH = len(CHUNKS)  # 8 chunks per head pass

    def norm_head(h, ctx_ps, ctxn):
        fq, rq = divmod(h * HD, P)
        sm = nrm.tile([1, 512], f32, tag="sm")
        nc.vector.tensor_copy(sm[:], ctx_ps[HD:HD + 1, :])
        sb = nrm.tile([HD, 512], f32, tag="sb")
        nc.gpsimd.partition_broadcast(sb[:], sm[:])
        rb = nrm.tile([HD, 512], f32, tag="rb")
        rs = nrm.tile([HD, 512], f32, tag="rs")
        nc.vector.reciprocal_approx_accurate(rb[:], sb[:], rs[:])
        nc.vector.tensor_mul(ctxn[fq][rq:rq + HD, :], ctx_ps[0:HD, :], rb[:])

    prev = None
    for j in range(NQ):
        queue = list(j0_fill) if j == 0 else list(jn_fill[j])
        n_early = len(queue)
        if prev is not None:
            queue += outproj_units(prev[0], prev[1])
        pops_per_chunk = 3 if j == 0 else 1
        skip_chunks = 0 if j == 0 else 2  # let the norm chain drain first
        qi = 0
        ctxn = [ctxp.tile([P, 512], bf16, tag=f"ctxn{f}", name=f"ctxn{f}")
                for f in range(FK)]
        ctx_ps_h = {}
        exb = {}           # (h, c) -> ex tile (deferred-consumption buffers)
        pending = []       # ctx chunk work: (h, c)
        done_ctx = {h: 0 for h in range(NHC)}

        def emit_ctx(h, c):
            if h not in ctx_ps_h:
                ctx_ps_h[h] = acc_ps.tile([VW, 512], f32, tag="acc",
                                          name=f"ctxps{h}")
            cp = ctx_ps_h[h]
            ex = exb.pop((h, c))
            for t in range(2):
                kt2 = 2 * c + t
                nc.tensor.matmul(
                    cp[:], vsb[kt2][:, h * VW:(h + 1) * VW],
                    ex[:, t * 512:(t + 1) * 512],
                    start=(kt2 == 0), stop=(kt2 == SK - 1))
            done_ctx[h] += 1
            if done_ctx[h] == NCH:
                norm_head(h, cp, ctxn)

        def vsb_ready_upto(popped):
            vp = max(0, popped - n_vp_before) if j == 0 else 10 ** 9
            return vp // 2 - 1 if j == 0 else 10 ** 9

        for h in range(NHC):
            fq, rq = divmod(h * HD, P)
            qv = qT[fq][rq:rq + HD, j * 512:(j + 1) * 512]
            for c in range(NCH):
                sc = sc_ps.tile([P, 2 * 512], f32, tag="sc")
                for t in range(2):
                    kt2 = 2 * c + t
                    nc.tensor.matmul(
                        sc[:, t * 512:(t + 1) * 512],
                        kT[fq][rq:rq + HD, kt2 * P:(kt2 + 1) * P],
                        qv, start=True, stop=True)
                ex = exp.tile([P, 2 * 512], bf16, tag="ex")
                nc.scalar.activation(ex[:], sc[:], EXP,
                                     scale=1.0 / (HD ** 0.5))
                exb[(h, c)] = ex
                pending.append((h, c))
                gchunk = h * NCH + c
                if gchunk >= skip_chunks or qi < n_early:
                    for _ in range(pops_per_chunk):
                        if qi < len(queue):
                            queue[qi]()
                            qi += 1
                tmax = vsb_ready_upto(qi)
                while pending and 2 * pending[0][1] + 1 <= tmax:
                    emit_ctx(*pending.pop(0))
        while qi < len(queue):
            queue[qi]()
            qi += 1
        while pending:
            emit_ctx(*pending.pop(0))
        prev = (j, ctxn)
    for u in outproj_units(prev[0], prev[1]):
        u()


_CACHE = {}


def _build():
    if "nc" in _CACHE:
        return _CACHE["nc"]
    nc = bacc.Bacc("TRN2", target_bir_lowering=False, debug=False)
    QT = nc.dram_tensor("QT", [D, S], bf16, kind="ExternalInput").ap()
    KT = nc.dram_tensor("KT", [D, S], bf16, kind="ExternalInput").ap()
    VT = nc.dram_tensor("VT", [D, S], bf16, kind="ExternalInput").ap()
    WqT = nc.dram_tensor("WqT", [D, FS], bf16, kind="ExternalInput").ap()
    WkT = nc.dram_tensor("WkT", [D, FS], bf16, kind="ExternalInput").ap()
    WvT = nc.dram_tensor("WvT", [D, FS], bf16, kind="ExternalInput").ap()
    WoT = nc.dram_tensor("WoT", [FS, D], bf16, kind="ExternalInput").ap()
    bq = nc.dram_tensor("bq", [FS, 1], f32, kind="ExternalInput").ap()
    bk = nc.dram_tensor("bk", [FS, 1], f32, kind="ExternalInput").ap()
    bv = nc.dram_tensor("bv", [1, FS], f32, kind="ExternalInput").ap()
    OUTP = nc.dram_tensor("OUTP", [S, D], f32, kind="ExternalOutput").ap()
    with tile.TileContext(nc) as tc, ExitStack() as ctx:
        _emit(ctx, tc, nc, (QT, KT, VT, WqT, WkT, WvT, WoT, bq, bk, bv, OUTP))
    nc.compile()
    _CACHE["nc"] = nc
    return nc


def _in_maps(Q, K, V, Wq, bq, Wk, bk, Wv, bv, Wo, bo):
    bf = ml_dtypes.bfloat16

    def cT(a):  # contiguous bf16 transpose
        return np.ascontiguousarray(np.asarray(a).T).astype(bf)

    QTb = [cT(Q[b]) for b in range(B)]
    KTb = [cT(K[b]) for b in range(B)]
    VTb = [cT(V[b]) for b in range(B)]
    c = np.ascontiguousarray
    maps = []
    for core in range(8):
        b, g = divmod(core, NG)
        sl = slice(g * FS, (g + 1) * FS)
        maps.append({
            "QT": QTb[b], "KT": KTb[b], "VT": VTb[b],
            "WqT": cT(Wq[sl, :]), "WkT": cT(Wk[sl, :]),
            "WvT": cT(Wv[sl, :]), "WoT": cT(Wo[:, sl]),
            "bq": c(np.asarray(bq)[sl].reshape(FS, 1)),
            "bk": c(np.asarray(bk)[sl].reshape(FS, 1)),
            "bv": c(np.asarray(bv)[sl].reshape(1, FS)),
        })
    return maps


def kernel(Q, K, V, Wq, bq, Wk, bk, Wv, bv, Wo, bo):
    nc = _build()
    maps = _in_maps(Q, K, V, Wq, bq, Wk, bk, Wv, bv, Wo, bo)
    res = run_bass_kernel_spmd(nc, maps, core_ids=list(range(8)))
    out = np.empty((B, S, D), np.float32)
    for b in range(B):
        acc = res.results[b * NG]["OUTP"].astype(np.float32)
        for g in range(1, NG):
            acc = acc + res.results[b * NG + g]["OUTP"]
        out[b] = acc + np.asarray(bo, np.float32)[None, :]
    return out
